# revision 1
# baseline (speedup 1.0000x reference)
"""MoE-routed dynamic conv kernel for Trainium2 (8 NeuronCores, SPMD).

Problem: per-sample attention (global avg pool -> 1x1 conv -> sigmoid) mixes
K=4 expert 3x3 conv kernels; each sample is convolved with its own mixed
kernel.  x: (32, 256, 56, 56), att_w: (4, 256), weight: (4, 256, 256, 3, 3).

Strategy: data parallel over batch (4 samples per core, weights replicated).
x is zero-padded to (58, 58) on the host, so on device every conv tap
(kh, kw) is a flat contiguous slice of the padded image.  Per sample:
  - pooled sums via DVE free-dim reduce over the padded x tile
  - attention logits via tiny f32 PE matmuls against a host-side replicated
    att_w (gives att_k broadcast across all 128 partitions), sigmoid on ACT
  - expert mixing (agg = sum_k att_k * w_k) via 4 fused DVE ops per ci-block
  - conv as implicit GEMM in fp32r (FP22-truncated reads, full PE rate,
    even-count/aligned APs per the fp32r ISA restrictions): 18 matmuls
    (9 taps x 2 ci-blocks) accumulate into each PSUM chunk of 464 output
    columns (8 rows x 58); the two padded columns per row are discarded by
    the strided output DMA.

The per-sample stages are software-pipelined (att/mix of sample b+1 is
emitted before the conv of sample b) so the PE never waits on the
attention -> sigmoid -> mixing chain at sample boundaries.
"""

import sys

if "/opt/trn_rl_repo" not in sys.path:
    sys.path.insert(0, "/opt/trn_rl_repo")

import numpy as np

B_TOTAL = 32
N_CORES = 8
B_PER_CORE = B_TOTAL // N_CORES  # 4
CI = 256
CO = 256
K = 4
H = W = 56
PH = PW = 58
FLAT = PH * PW            # 3364 padded image
XT_F = FLAT + 4           # 3368: + tail pad for tap (2,2) overrun, host zeros
OUTF = H * W              # 3136 output cols per co-block (contiguous)
RPC = 8                   # output rows per PSUM chunk
NCHUNK = RPC * W          # 448 = 8 rows x 56 valid cols (even, aligned)
NCHUNKS = H // RPC        # 7
TAPS = 9
TPC = TAPS * CO           # 2304 free elems per (k, ci-block) weight tile

_cache = {}


def _build_nc():
    from contextlib import ExitStack

    import concourse.bacc as bacc
    import concourse.mybir as mybir
    import concourse.tile as tile

    f32 = mybir.dt.float32
    f32r = mybir.dt.float32r
    AF = mybir.ActivationFunctionType
    ALU = mybir.AluOpType

    nc = bacc.Bacc("TRN2", target_bir_lowering=False, debug=False)
    x_p = nc.declare_dram_parameter("x", [B_PER_CORE, CI, XT_F], f32r, isOutput=False)
    w_p = nc.declare_dram_parameter("w", [K, CI, 3, 3, CO], f32, isOutput=False)
    ar_p = nc.declare_dram_parameter("attrep", [CI, K * 128], f32, isOutput=False)
    o_p = nc.declare_dram_parameter("out", [B_PER_CORE, CO, H, W], f32, isOutput=True)

    with ExitStack() as ctx:
        tc = ctx.enter_context(tile.TileContext(nc))
        pw = ctx.enter_context(tc.tile_pool(name="wpool", bufs=1))
        px = ctx.enter_context(tc.tile_pool(name="xpool", bufs=4))
        pagg = ctx.enter_context(tc.tile_pool(name="aggpool", bufs=4))
        pout = ctx.enter_context(tc.tile_pool(name="outpool", bufs=2))
        psml = ctx.enter_context(tc.tile_pool(name="small", bufs=4))
        pps = ctx.enter_context(tc.tile_pool(name="cpsum", bufs=7, space="PSUM"))
        ppsa = ctx.enter_context(tc.tile_pool(name="apsum", bufs=1, space="PSUM"))

        # Replicated attention weights (col j of block k = att_w[k, :]) and
        # the resident expert weights, free layout (k, tap, co) per ci-block.
        ar_sb = []
        for c in range(2):
            at = pw.tile([128, K * 128], f32, tag=f"ar{c}")
            nc.sync.dma_start(out=at[:, :], in_=ar_p[c * 128 : (c + 1) * 128, :])
            ar_sb.append(at)
        w_sb = [
            pw.tile([128, K * TPC], f32, tag=f"w{c}", name=f"wt{c}")
            for c in range(2)
        ]
        for k in range(K):
            for c in range(2):
                nc.sync.dma_start(
                    out=w_sb[c][:, k * TPC : (k + 1) * TPC],
                    in_=w_p[k, c * 128 : (c + 1) * 128].rearrange(
                        "ci kh kw co -> ci (kh kw co)"
                    ),
                )

        state = {}

        def stage_load(b):
            """Load padded x_b (both HWDGE engines) and pool."""
            xts = []
            pooleds = []
            for c in range(2):
                xt = px.tile([128, XT_F], f32r, tag="x")
                eng = nc.scalar if c == 0 else nc.gpsimd
                eng.dma_start(out=xt[:, :], in_=x_p[b, c * 128 : (c + 1) * 128, :])
                pl = psml.tile([128, 1], f32, tag="pooled")
                nc.vector.tensor_reduce(
                    pl[:, :], xt[:, 0:FLAT], axis=mybir.AxisListType.X, op=ALU.add
                )
                xts.append(xt)
                pooleds.append(pl)
            state[b] = (xts, pooleds)

        def stage_att(b):
            """Attention matmuls + sigmoid for sample b."""
            xts, pooleds = state[b]
            # att[k] broadcast over all partitions: lhsT column j = att_w[k,:]
            # for every j, so out[j, 0] = dot(att_w[k], pooled) for all j.
            # Plain f32 matmuls (fp32r forbids odd moving counts like N=1).
            att_ps = ppsa.tile([128, K], f32, tag="attps")
            for k in range(K):
                for c in range(2):
                    nc.tensor.matmul(
                        att_ps[:, k : k + 1],
                        lhsT=ar_sb[c][:, k * 128 : (k + 1) * 128],
                        rhs=pooleds[c][:, :],
                        start=(c == 0),
                        stop=(c == 1),
                    )
            att_sb = psml.tile([128, K], f32, tag="attsb")
            nc.scalar.activation(
                att_sb[:, :], att_ps[:, :], AF.Sigmoid, scale=1.0 / (H * W)
            )
            state[b] = (xts, att_sb)

        def stage_mix(b):
            """Expert mixing on DVE: agg = sum_k att_k * w_k, fused mul-add."""
            xts, att_sb = state[b]
            aggs = []
            for c in range(2):
                ag = pagg.tile([128, TPC], f32r, tag="agg")
                nc.vector.tensor_scalar_mul(ag[:, :], w_sb[c][:, 0:TPC], att_sb[:, 0:1])
                for k in range(1, K):
                    nc.vector.scalar_tensor_tensor(
                        ag[:, :],
                        w_sb[c][:, k * TPC : (k + 1) * TPC],
                        att_sb[:, k : k + 1],
                        ag[:, :],
                        ALU.mult,
                        ALU.add,
                    )
                aggs.append(ag)
            state[b] = (xts, aggs)


        def stage_b(b, after_first_chunk=None):
            """Conv for sample b: per co-block, 7 PSUM chunks of 8x56 cols.

            Each tap is a 2D window [8 rows, 56 valid cols] of the padded
            image (row stride 58), written to a dense [8, 56] PSUM chunk:
            no wasted pad columns, and the output buffer stays contiguous.
            """
            xts, aggs = state.pop(b)
            x3s = [
                xt[:, :FLAT].rearrange("p (h w) -> p h w", h=PH) for xt in xts
            ]
            first_chunk_done = False
            for cb in range(2):
                osb = pout.tile([128, OUTF], f32, tag="osb")
                for ch in range(NCHUNKS):
                    r0 = ch * RPC
                    ps = pps.tile([128, NCHUNK], f32, tag="convps")
                    ps3 = ps[:, :].rearrange("p (h w) -> p h w", h=RPC)
                    i = 0
                    for c in range(2):
                        for t in range(TAPS):
                            dr, dc = t // 3, t % 3
                            nc.tensor.matmul(
                                ps3[:, :, :],
                                lhsT=aggs[c][
                                    :, t * CO + cb * 128 : t * CO + cb * 128 + 128
                                ],
                                rhs=x3s[c][
                                    :, r0 + dr : r0 + dr + RPC, dc : dc + W
                                ],
                                start=(i == 0),
                                stop=(i == 17),
                            )
                            i += 1
                    nc.scalar.copy(osb[:, ch * NCHUNK : (ch + 1) * NCHUNK], ps[:, :])
                    if not first_chunk_done:
                        first_chunk_done = True
                        if after_first_chunk is not None:
                            after_first_chunk()
                # Contiguous output; split in halves so the first transfer
                # overlaps the remaining evictions.
                half = OUTF // 2
                for s in range(2):
                    nc.sync.dma_start(
                        out=o_p[b, cb * 128 : (cb + 1) * 128].rearrange(
                            "co h w -> co (h w)"
                        )[:, s * half : (s + 1) * half],
                        in_=osb[:, s * half : (s + 1) * half],
                    )

        # Software pipeline: loads lead their mix; att/mix of b+1 precedes
        # conv of b so the PE never stalls on the attention chain.
        stage_load(0)
        stage_att(0)
        stage_mix(0)
        stage_load(1)

        def _att_mix_1():
            stage_att(1)
            stage_mix(1)

        stage_b(0, after_first_chunk=_att_mix_1)
        stage_load(2)
        stage_att(2)
        stage_mix(2)
        stage_b(1)
        stage_load(3)
        stage_att(3)
        stage_mix(3)
        stage_b(2)
        stage_b(3)

    nc.compile()
    return nc


def _get_nc():
    if "nc" not in _cache:
        _cache["nc"] = _build_nc()
    return _cache["nc"]


def _make_in_maps(x, att_w, weight):
    x = np.asarray(x, dtype=np.float32)
    att_w = np.asarray(att_w, dtype=np.float32)
    weight = np.asarray(weight, dtype=np.float32)
    # Host-side zero pad to (58, 58) + 4 tail elems, flattened per channel.
    xp = np.zeros((B_TOTAL, CI, XT_F), dtype=np.float32)
    xp[:, :, :FLAT] = np.pad(
        x, ((0, 0), (0, 0), (1, 1), (1, 1))
    ).reshape(B_TOTAL, CI, FLAT)
    # (K, Cout, Cin, kh, kw) -> (K, Cin, kh, kw, Cout) so the SBUF lhsT
    # layout [ci, (tap, co)] is a contiguous DMA.
    w_t = np.ascontiguousarray(weight.transpose(0, 2, 3, 4, 1))
    # (Cin, K*128): col j = att_w[j // 128, ci]
    att_rep = np.ascontiguousarray(np.repeat(att_w.T, 128, axis=1))
    return [
        {
            "x": np.ascontiguousarray(xp[i * B_PER_CORE : (i + 1) * B_PER_CORE]),
            "w": w_t,
            "attrep": att_rep,
        }
        for i in range(N_CORES)
    ]


def _run(x, att_w, weight, trace=False, **spmd_kwargs):
    from concourse.bass_utils import run_bass_kernel_spmd

    nc = _get_nc()
    in_maps = _make_in_maps(x, att_w, weight)
    res = run_bass_kernel_spmd(
        nc, in_maps, list(range(N_CORES)), trace=trace, **spmd_kwargs
    )
    out = np.concatenate([r["out"] for r in res.results], axis=0)
    return out.astype(np.float32, copy=False), res


def kernel(x, att_w, weight):
    out, _ = _run(x, att_w, weight)
    return out



# revision 9
# speedup vs baseline: 1.1691x; 1.1691x over previous
"""MoE-routed dynamic conv kernel for Trainium2 (8 NeuronCores, SPMD).

Problem: per-sample attention (global avg pool -> 1x1 conv -> sigmoid) mixes
K=4 expert 3x3 conv kernels; each sample is convolved with its own mixed
kernel.  x: (32, 256, 56, 56), att_w: (4, 256), weight: (4, 256, 256, 3, 3).

Strategy: data parallel over batch (4 samples per core, weights replicated),
with the conv computed as 1-D Winograd F(2,3) along W (direct along H):
  - x is zero-padded to (58, 58) on the host, cast to bf16 and split into
    even/odd column-parity planes so every device op streams step-1.
  - the K expert banks are pre-transformed on the host along kw with
    G = [[1,0,0],[.5,.5,.5],[.5,-.5,.5],[0,0,1]] -> layout (Cin, K, j, kh, Co).
  - attention: free-dim pool sums on GPSIMD+DVE, logits via a GPSIMD
    partition all-reduce of att_w * pooled, sigmoid on ACT.  No PSUM used,
    so the conv owns all 8 banks.
  - expert mixing agg = sum_k att_k * w_k in bf16: two products as DVE
    tensor_scalar (4x mode), two as ACT activation-copy with per-partition
    scale, then an in-place DVE add chain (2x mode).
  - input transform D_j = B^T-combo of the parity planes: 4 DVE
    tensor_tensor ops per ci-block (bf16 2x).
  - GEMM: per (co-block, 14-row chunk): 4 j-planes x (3 kh x 2 ci-blocks)
    bf16 matmuls accumulate N=392 columns into bank-aligned PSUM planes.
  - eviction: one ACT copy per chunk moves the 4 planes to SBUF as bf16;
    DVE collapses them (Ye = M0+M1+M2, Yo = M1-M2-M3) into parity output
    planes which DMA out as bf16; the host interleaves parities and
    upcasts to f32.

The per-sample stages are software-pipelined (att is computed two samples
ahead, D/mix one sample ahead) so the PE never waits on the attention ->
mixing chain at sample boundaries.
"""

import sys

if "/opt/trn_rl_repo" not in sys.path:
    sys.path.insert(0, "/opt/trn_rl_repo")

import numpy as np

B_TOTAL = 32
N_CORES = 8
BPC = B_TOTAL // N_CORES  # 4
CI = 256
CO = 256
K = 4
H = W = 56
PH = 58                   # padded rows
TWP = 29                  # parity-plane cols (58/2)
TW = 28                   # output tiles per row (W/2)
NJ = 4                    # wino points
NDR = 3                   # kh taps
XF = 2 * PH * TWP         # 3364 x elems per channel (par, h, twp)
WF = NJ * NDR * CO        # 3072 wino weight elems per (k, ci): (j, dr, co)
DF = NJ * PH * TW         # 6496 D elems per channel (j, h, tw)
CHR = 14                  # oh rows per PSUM chunk
NCH = H // CHR            # 4 chunks
NC_ = CHR * TW            # 392 cols per chunk (<= 512 psum bank)
OF = H * TW               # 1568 out elems per parity per co-block

_cache = {}


def _build_nc():
    from contextlib import ExitStack

    import concourse.bacc as bacc
    import concourse.bass_isa as bass_isa
    import concourse.mybir as mybir
    import concourse.tile as tile

    f32 = mybir.dt.float32
    bf16 = mybir.dt.bfloat16
    AF = mybir.ActivationFunctionType
    ALU = mybir.AluOpType
    AX = mybir.AxisListType

    nc = bacc.Bacc("TRN2", target_bir_lowering=False, debug=False)
    x_p = nc.declare_dram_parameter("x", [BPC, CI, XF], bf16, isOutput=False)
    w_p = nc.declare_dram_parameter("w", [CI, K * WF], bf16, isOutput=False)
    aw_p = nc.declare_dram_parameter("aw", [CI, K], f32, isOutput=False)
    o_p = nc.declare_dram_parameter("out", [BPC, 2, CO, OF], bf16, isOutput=True)

    with ExitStack() as ctx:
        tc = ctx.enter_context(tile.TileContext(nc))
        pw = ctx.enter_context(tc.tile_pool(name="wpool", bufs=1))
        px = ctx.enter_context(tc.tile_pool(name="xpool", bufs=2))
        pd = ctx.enter_context(tc.tile_pool(name="dpool", bufs=2))
        pagg = ctx.enter_context(tc.tile_pool(name="aggpool", bufs=2))
        ptmp = ctx.enter_context(tc.tile_pool(name="mixtmp", bufs=1))
        pu = ctx.enter_context(tc.tile_pool(name="mixu", bufs=2))
        pm = ctx.enter_context(tc.tile_pool(name="mpool", bufs=3))
        psd = ctx.enter_context(tc.tile_pool(name="sdpool", bufs=2))
        py = ctx.enter_context(tc.tile_pool(name="ypool", bufs=2))
        psml = ctx.enter_context(tc.tile_pool(name="small", bufs=3))
        pps = ctx.enter_context(tc.tile_pool(name="cpsum", bufs=2, space="PSUM"))

        # Resident replicated weights: wino expert banks + attention weights.
        w_sb = []
        aw_sb = []
        for c in range(2):
            wt = pw.tile([128, K * WF], bf16, tag=f"w{c}")
            nc.sync.dma_start(out=wt[:, :], in_=w_p[c * 128 : (c + 1) * 128, :])
            w_sb.append(wt)
            at = pw.tile([128, K], f32, tag=f"aw{c}")
            nc.sync.dma_start(out=at[:, :], in_=aw_p[c * 128 : (c + 1) * 128, :])
            aw_sb.append(at)

        xts = {}
        atts = {}
        aggs = {}
        dts = {}
        pools = {}

        def stage_load(b):
            xb = []
            for c in range(2):
                xt = px.tile([128, XF], bf16, tag=f"x{c}")
                eng = nc.sync if c == 0 else nc.scalar
                eng.dma_start(out=xt[:, :], in_=x_p[b, c * 128 : (c + 1) * 128, :])
                xb.append(xt)
            xts[b] = xb

        def stage_att(b):
            """pooled (from stage_d accum) -> logits (partition all-reduce)
            -> sigmoid."""
            pooled = pools.pop(b)
            tka = psml.tile([128, K], f32, tag="tka")
            tkb = psml.tile([128, K], f32, tag="tkb")
            nc.vector.tensor_scalar_mul(tka[:, :], aw_sb[0][:, :], pooled[0][:, :])
            nc.vector.tensor_scalar_mul(tkb[:, :], aw_sb[1][:, :], pooled[1][:, :])
            nc.vector.tensor_tensor(tka[:, :], tka[:, :], tkb[:, :], ALU.add)
            logit = psml.tile([128, K], f32, tag="logit")
            nc.gpsimd.partition_all_reduce(
                logit[:, :], tka[:, :], 128, bass_isa.ReduceOp.add
            )
            att = psml.tile([128, K], f32, tag="att")
            nc.scalar.activation(
                att[:, :], logit[:, :], AF.Sigmoid, scale=1.0 / (H * W)
            )
            atts[b] = att

        def stage_d(b):
            """Input transform: D_j from parity planes, 4 ops per ci-block.

            The d1 op runs as scalar_tensor_tensor with accum_out: its free
            sum equals the original per-channel spatial sum exactly (the pad
            columns are zero), so the attention pool comes for free.
            """
            xb = xts.pop(b)
            db = []
            pl = []
            for c in range(2):
                x3 = xb[c][:, :].rearrange("p (q h t) -> p q h t", q=2, h=PH)
                xe0 = x3[:, 0, :, 0:TW]
                xe1 = x3[:, 0, :, 1 : TW + 1]
                xo0 = x3[:, 1, :, 0:TW]
                xo1 = x3[:, 1, :, 1 : TW + 1]
                dt = pd.tile([128, DF], bf16, tag=f"d{c}")
                d3 = dt[:, :].rearrange("p (j h t) -> p j h t", j=NJ, h=PH)
                pc = psml.tile([128, 1], f32, tag=f"pooled{c}")
                nc.vector.tensor_tensor(d3[:, 0], xe0, xe1, ALU.subtract)
                nc.vector.scalar_tensor_tensor(
                    d3[:, 1], xo0, 1.0, xe1, ALU.mult, ALU.add, accum_out=pc[:, :]
                )
                nc.vector.tensor_tensor(d3[:, 2], xe1, xo0, ALU.subtract)
                nc.vector.tensor_tensor(d3[:, 3], xo0, xo1, ALU.subtract)
                db.append(dt)
                pl.append(pc)
            dts[b] = db
            pools[b] = pl

        def stage_mix(b):
            """agg_c = sum_k att_k * w_k; 2 products on ACT, 2 on DVE."""
            att = atts.pop(b)
            ab = []
            for c in range(2):
                ag = pagg.tile([128, WF], bf16, tag=f"agg{c}")
                t = ptmp.tile([128, WF], bf16, tag=f"t{c}")
                u0 = pu.tile([128, WF], bf16, tag="u")
                u1 = pu.tile([128, WF], bf16, tag="u")
                nc.scalar.activation(
                    u0[:, :], w_sb[c][:, 2 * WF : 3 * WF], AF.Copy,
                    scale=att[:, 2:3],
                )
                nc.scalar.activation(
                    u1[:, :], w_sb[c][:, 3 * WF : 4 * WF], AF.Copy,
                    scale=att[:, 3:4],
                )
                nc.vector.tensor_scalar_mul(ag[:, :], w_sb[c][:, 0:WF], att[:, 0:1])
                nc.vector.tensor_scalar_mul(t[:, :], w_sb[c][:, WF : 2 * WF], att[:, 1:2])
                nc.vector.tensor_tensor(ag[:, :], ag[:, :], t[:, :], ALU.add)
                nc.vector.tensor_tensor(ag[:, :], ag[:, :], u0[:, :], ALU.add)
                nc.vector.tensor_tensor(ag[:, :], ag[:, :], u1[:, :], ALU.add)
                ab.append(ag)
            aggs[b] = ab

        def stage_conv(b):
            """GEMM + eviction (PE + ACT); returns collapse closures for DVE."""
            db = dts.pop(b)
            ab = aggs.pop(b)
            d3s = [
                dt[:, :].rearrange("p (j h t) -> p j h t", j=NJ, h=PH) for dt in db
            ]
            collapse = []
            for cb in range(2):
                yt = py.tile([128, 2 * OF], bf16, tag="y")
                y4 = yt[:, :].rearrange("p (q h t) -> p q h t", q=2, h=H)
                for ch in range(NCH):
                    r0 = ch * CHR
                    ps = pps.tile([128, NJ * 512], f32, tag="convps")
                    for j in range(NJ):
                        out3 = ps[:, j * 512 : j * 512 + NC_].rearrange(
                            "p (h t) -> p h t", h=CHR
                        )
                        i = 0
                        for dr in range(NDR):
                            for c in range(2):
                                nc.tensor.matmul(
                                    out3[:, :, :],
                                    lhsT=ab[c][
                                        :,
                                        (j * NDR + dr) * CO
                                        + cb * 128 : (j * NDR + dr) * CO
                                        + cb * 128
                                        + 128,
                                    ],
                                    rhs=d3s[c][:, j, r0 + dr : r0 + dr + CHR, :],
                                    start=(i == 0),
                                    stop=(i == 2 * NDR - 1),
                                )
                                i += 1
                    mt = pm.tile([128, NJ * NC_], bf16, tag="m")
                    nc.scalar.copy(
                        out=mt[:, :].rearrange("p (j n) -> p j n", j=NJ),
                        in_=ps[:, :].rearrange("p (j n) -> p j n", j=NJ)[
                            :, :, 0:NC_
                        ],
                    )
                    collapse.append((cb, ch, yt, y4, mt))
                # DMA out is emitted by the collapse closure below.
            return collapse

        def stage_collapse(b, items):
            """DVE j-collapse into parity planes + output DMA."""
            for cb, ch, yt, y4, mt in items:
                m3 = mt[:, :].rearrange("p (j h t) -> p j h t", j=NJ, h=CHR)
                s = psd.tile([128, NC_], bf16, tag="s")
                d = psd.tile([128, NC_], bf16, tag="d")
                s3 = s[:, :].rearrange("p (h t) -> p h t", h=CHR)
                d3 = d[:, :].rearrange("p (h t) -> p h t", h=CHR)
                r0 = ch * CHR
                nc.vector.tensor_tensor(s3, m3[:, 1], m3[:, 2], ALU.add)
                nc.vector.tensor_tensor(d3, m3[:, 1], m3[:, 2], ALU.subtract)
                nc.vector.tensor_tensor(
                    y4[:, 0, r0 : r0 + CHR, :], m3[:, 0], s3, ALU.add
                )
                nc.vector.tensor_tensor(
                    y4[:, 1, r0 : r0 + CHR, :], d3, m3[:, 3], ALU.subtract
                )
                if ch == NCH - 1:
                    for par in range(2):
                        nc.sync.dma_start(
                            out=o_p[b, par, cb * 128 : (cb + 1) * 128, :],
                            in_=yt[:, :].rearrange(
                                "p (q f) -> p q f", q=2
                            )[:, par, :],
                        )

        # ---- software pipeline ----
        stage_load(0)
        stage_load(1)
        stage_d(0)
        stage_att(0)
        stage_mix(0)
        for b in range(BPC):
            if b + 2 < BPC:
                stage_load(b + 2)
            if b + 1 < BPC:
                stage_d(b + 1)
                stage_att(b + 1)
                stage_mix(b + 1)
            items = stage_conv(b)
            stage_collapse(b, items)

    nc.compile()
    return nc


def _get_nc():
    if "nc" not in _cache:
        _cache["nc"] = _build_nc()
    return _cache["nc"]


def _make_in_maps(x, att_w, weight):
    from ml_dtypes import bfloat16

    x = np.asarray(x, dtype=np.float32)
    att_w = np.asarray(att_w, dtype=np.float32)
    weight = np.asarray(weight, dtype=np.float32)
    # pad to (58, 58), split w-parity, cast bf16: (B, CI, 2, 58, 29)
    xp = np.pad(x, ((0, 0), (0, 0), (1, 1), (1, 1)))
    xh = np.empty((B_TOTAL, CI, 2, PH, TWP), dtype=bfloat16)
    xh[:, :, 0] = xp[:, :, :, 0::2]
    xh[:, :, 1] = xp[:, :, :, 1::2]
    xh = xh.reshape(B_TOTAL, CI, XF)
    # wino expert banks: (K, Cout, Cin, kh, kw) -> (Cin, K, j, kh, Cout)
    G = np.array(
        [[1, 0, 0], [0.5, 0.5, 0.5], [0.5, -0.5, 0.5], [0, 0, 1]], np.float32
    )
    wj = np.einsum("jd,koihd->ikjho", G, weight)
    wj = np.ascontiguousarray(wj).astype(bfloat16).reshape(CI, K * WF)
    awt = np.ascontiguousarray(att_w.T)  # (CI, K) f32
    return [
        {
            "x": np.ascontiguousarray(xh[i * BPC : (i + 1) * BPC]),
            "w": wj,
            "aw": awt,
        }
        for i in range(N_CORES)
    ]


def _run(x, att_w, weight, trace=False, **spmd_kwargs):
    from concourse.bass_utils import run_bass_kernel_spmd

    nc = _get_nc()
    in_maps = _make_in_maps(x, att_w, weight)
    res = run_bass_kernel_spmd(
        nc, in_maps, list(range(N_CORES)), trace=trace, **spmd_kwargs
    )
    o = np.concatenate([r["out"] for r in res.results], axis=0)
    # (B, 2, CO, H*TW) bf16 -> interleave parities, upcast
    o = o.reshape(B_TOTAL, 2, CO, H, TW).astype(np.float32)
    out = np.empty((B_TOTAL, CO, H, W), dtype=np.float32)
    out[:, :, :, 0::2] = o[:, 0]
    out[:, :, :, 1::2] = o[:, 1]
    return out, res


def kernel(x, att_w, weight):
    out, _ = _run(x, att_w, weight)
    return out


# revision 18
# speedup vs baseline: 1.2817x; 1.0964x over previous
"""MoE-routed dynamic conv kernel for Trainium2 (8 NeuronCores, SPMD).

Problem: per-sample attention (global avg pool -> 1x1 conv -> sigmoid) mixes
K=4 expert 3x3 conv kernels; each sample is convolved with its own mixed
kernel.  x: (32, 256, 56, 56), att_w: (4, 256), weight: (4, 256, 256, 3, 3).

Strategy: data parallel over batch (4 samples per core, weights replicated),
with the conv computed as 1-D Winograd F(2,3) along W (direct along H), all
tensors bf16 on the wire and in SBUF (fp32 PSUM accumulation):
  - x is zero-padded to (58, 58) on the host, cast to bf16 and split into
    even/odd column-parity planes so every device op streams step-1.
  - the K expert banks are pre-transformed on the host along kw with
    G = [[1,0,0],[.5,.5,.5],[.5,-.5,.5],[0,0,1]] -> layout (Cin, K, j, kh, Co).
  - attention (2 samples ahead): pooled sums via ACT accumulator, logits via
    a GPSIMD partition all-reduce of att_w * pooled, sigmoid on ACT.  No
    PSUM used, so the conv owns all 8 banks.
  - expert mixing (1 sample ahead): the 4 products att_k * w_k run on ACT
    as activation-copies with per-partition scale; DVE folds them with an
    in-place add chain (bf16 2x mode).
  - input transform (1 sample ahead): D_j = B^T-combo of the parity planes,
    4 DVE tensor_tensor ops per ci-block (bf16 2x).
  - GEMM: per (co-block, 14-row chunk): 4 j-planes x (3 kh x 2 ci-blocks)
    bf16 matmuls accumulate N=392 columns into bank-aligned PSUM planes
    (two 4-bank chunk tiles ping-pong).
  - eviction: one GPSIMD copy per chunk moves the 4 planes to SBUF as bf16;
    DVE collapses them (Ye = M0+M1+M2, Yo = M1-M2-M3) into parity output
    planes which DMA out as bf16; the host interleaves parities and
    upcasts to f32.

Engine-queue emission order is chosen so every engine's in-order queue sees
work in the order it becomes ready (PAR ahead of evictions on GPSIMD, mix
products ahead of nothing on ACT, D/mix ahead of collapse on DVE), keeping
the PE fed back-to-back.
"""

import sys

if "/opt/trn_rl_repo" not in sys.path:
    sys.path.insert(0, "/opt/trn_rl_repo")

import numpy as np

B_TOTAL = 32
N_CORES = 8
BPC = B_TOTAL // N_CORES  # 4
CI = 256
CO = 256
K = 4
H = W = 56
PH = 58                   # padded rows
TWP = 29                  # parity-plane cols (58/2)
TW = 28                   # output tiles per row (W/2)
NJ = 4                    # wino points
NDR = 3                   # kh taps
XF = 2 * PH * TWP         # 3364 x elems per channel (par, h, twp)
WF = NJ * NDR * CO        # 3072 wino weight elems per (k, ci): (j, dr, co)
DF = NJ * PH * TW         # 6496 D elems per channel (j, h, tw)
CHR = 14                  # oh rows per PSUM chunk
NCH = H // CHR            # 4 chunks
NC_ = CHR * TW            # 392 cols per chunk (<= 512 psum bank)
OF = H * TW               # 1568 out elems per parity per co-block

_cache = {}


def _build_nc():
    from contextlib import ExitStack

    import concourse.bacc as bacc
    import concourse.bass_isa as bass_isa
    import concourse.mybir as mybir
    import concourse.tile as tile

    f32 = mybir.dt.float32
    bf16 = mybir.dt.bfloat16
    AF = mybir.ActivationFunctionType
    ALU = mybir.AluOpType

    nc = bacc.Bacc("TRN2", target_bir_lowering=False, debug=False)
    x_p = nc.declare_dram_parameter("x", [BPC, CI, XF], bf16, isOutput=False)
    w_p = nc.declare_dram_parameter("w", [CI, K, WF], bf16, isOutput=False)
    aw_p = nc.declare_dram_parameter("aw", [CI, K], f32, isOutput=False)
    o_p = nc.declare_dram_parameter("out", [BPC, 2, CO, OF], bf16, isOutput=True)

    with ExitStack() as ctx:
        tc = ctx.enter_context(tile.TileContext(nc))
        pw = ctx.enter_context(tc.tile_pool(name="wpool", bufs=1))
        px = ctx.enter_context(tc.tile_pool(name="xpool", bufs=2))
        pd = ctx.enter_context(tc.tile_pool(name="dpool", bufs=2))
        pagg = ctx.enter_context(tc.tile_pool(name="aggpool", bufs=2))
        pu = ctx.enter_context(tc.tile_pool(name="mixu", bufs=3))
        put1 = ctx.enter_context(tc.tile_pool(name="mixt1", bufs=2))
        pm = ctx.enter_context(tc.tile_pool(name="mpool", bufs=2))
        psd = ctx.enter_context(tc.tile_pool(name="sdpool", bufs=2))
        py = ctx.enter_context(tc.tile_pool(name="ypool", bufs=2))
        psml = ctx.enter_context(tc.tile_pool(name="small", bufs=3))
        pdump = ctx.enter_context(tc.tile_pool(name="dump", bufs=1))
        pps = ctx.enter_context(tc.tile_pool(name="cpsum", bufs=2, space="PSUM"))

        # Resident replicated weights, loaded per (ci-block, expert) so the
        # first mixing products can start before the whole bank lands.
        # DMA-queue order interleaves with the x(0) load (emitted by the
        # pipeline below before load_weights is called).
        aw_sb = [
            pw.tile([128, K], f32, tag=f"aw{c}", name=f"aw{c}") for c in range(2)
        ]
        w_sb = [
            pw.tile([128, K * WF], bf16, tag=f"w{c}", name=f"wt{c}")
            for c in range(2)
        ]

        def load_weights():
            for c in range(2):
                nc.sync.dma_start(
                    out=aw_sb[c][:, :], in_=aw_p[c * 128 : (c + 1) * 128, :]
                )
            for k in range(K):
                for c in range(2):
                    nc.sync.dma_start(
                        out=w_sb[c][:, k * WF : (k + 1) * WF],
                        in_=w_p[c * 128 : (c + 1) * 128, k, :],
                    )

        xts = {}
        atts = {}
        aggs = {}
        dts = {}
        pools = {}

        def stage_load(b):
            xb = []
            for c in range(2):
                xt = px.tile([128, XF], bf16, tag=f"x{c}")
                nc.sync.dma_start(
                    out=xt[:, :], in_=x_p[b, c * 128 : (c + 1) * 128, :]
                )
                xb.append(xt)
            xts[b] = xb

        def stage_pool(b):
            """Pooled channel sums on ACT (accumulator), emitted right after
            the x DMA so it runs as the data lands."""
            xb = xts[b]
            pl = []
            for c in range(2):
                pc = psml.tile([128, 1], f32, tag=f"pooled{c}")
                dump = pdump.tile([128, XF], bf16, tag="pooldump")
                nc.scalar.activation(
                    dump[:, :], xb[c][:, :], AF.Copy, accum_out=pc[:, :]
                )
                pl.append(pc)
            pools[b] = pl

        def stage_att_reduce(b):
            """pooled -> logits via GPSIMD partition all-reduce -> sigmoid."""
            pooled = pools.pop(b)
            tka = psml.tile([128, K], f32, tag="tka")
            tkb = psml.tile([128, K], f32, tag="tkb")
            nc.vector.tensor_scalar_mul(tka[:, :], aw_sb[0][:, :], pooled[0][:, :])
            nc.vector.tensor_scalar_mul(tkb[:, :], aw_sb[1][:, :], pooled[1][:, :])
            nc.vector.tensor_tensor(tka[:, :], tka[:, :], tkb[:, :], ALU.add)
            logit = psml.tile([128, K], f32, tag="logit")
            nc.gpsimd.partition_all_reduce(
                logit[:, :], tka[:, :], 128, bass_isa.ReduceOp.add
            )
            att = psml.tile([128, K], f32, tag="att")
            nc.scalar.activation(
                att[:, :], logit[:, :], AF.Sigmoid, scale=1.0 / (H * W)
            )
            atts[b] = att

        def stage_d(b):
            """Input transform: D_j from parity planes, 4 TT per ci-block."""
            xb = xts.pop(b)
            db = []
            for c in range(2):
                x3 = xb[c][:, :].rearrange("p (q h t) -> p q h t", q=2, h=PH)
                xe0 = x3[:, 0, :, 0:TW]
                xe1 = x3[:, 0, :, 1 : TW + 1]
                xo0 = x3[:, 1, :, 0:TW]
                xo1 = x3[:, 1, :, 1 : TW + 1]
                dt = pd.tile([128, DF], bf16, tag=f"d{c}")
                d3 = dt[:, :].rearrange("p (j h t) -> p j h t", j=NJ, h=PH)
                nc.vector.tensor_tensor(d3[:, 0], xe0, xe1, ALU.subtract)
                nc.vector.tensor_tensor(d3[:, 1], xo0, xe1, ALU.add)
                nc.vector.tensor_tensor(d3[:, 2], xe1, xo0, ALU.subtract)
                nc.vector.tensor_tensor(d3[:, 3], xo0, xo1, ALU.subtract)
                db.append(dt)
            dts[b] = db

        def stage_mix(b):
            """agg_c = sum_k att_k * w_k.

            Products: ci-block 0 takes all four on ACT; ci-block 1 takes
            k0/k1 as DVE tensor_scalar (4x mode) and k2/k3 on ACT.  DVE then
            folds with in-place adds (bf16 2x), ordered by readiness.
            """
            att = atts.pop(b)

            def wslice(c, k):
                return w_sb[c][:, k * WF : (k + 1) * WF]

            def act_prod(dst, c, k):
                nc.scalar.activation(
                    dst[:, :], wslice(c, k), AF.Copy, scale=att[:, k : k + 1]
                )

            ag0 = pagg.tile([128, WF], bf16, tag="agg0")
            ag1 = pagg.tile([128, WF], bf16, tag="agg1")
            t1 = put1.tile([128, WF], bf16, tag="t1")
            u1c0 = pu.tile([128, WF], bf16, tag="u")
            u2c0 = pu.tile([128, WF], bf16, tag="u")
            u2c1 = pu.tile([128, WF], bf16, tag="u")
            u3c0 = pu.tile([128, WF], bf16, tag="u")
            u3c1 = pu.tile([128, WF], bf16, tag="u")
            # ACT queue (k-major so the preamble's per-k weight DMAs feed it)
            act_prod(ag0, 0, 0)
            act_prod(u1c0, 0, 1)
            act_prod(u2c0, 0, 2)
            act_prod(u2c1, 1, 2)
            act_prod(u3c0, 0, 3)
            act_prod(u3c1, 1, 3)
            # DVE products for ci-block 1
            nc.vector.tensor_scalar_mul(ag1[:, :], wslice(1, 0), att[:, 0:1])
            nc.vector.tensor_scalar_mul(t1[:, :], wslice(1, 1), att[:, 1:2])
            # DVE folds, readiness order
            TT = nc.vector.tensor_tensor
            TT(ag1[:, :], ag1[:, :], t1[:, :], ALU.add)
            TT(ag0[:, :], ag0[:, :], u1c0[:, :], ALU.add)
            TT(ag0[:, :], ag0[:, :], u2c0[:, :], ALU.add)
            TT(ag1[:, :], ag1[:, :], u2c1[:, :], ALU.add)
            TT(ag0[:, :], ag0[:, :], u3c0[:, :], ALU.add)
            TT(ag1[:, :], ag1[:, :], u3c1[:, :], ALU.add)
            aggs[b] = [ag0, ag1]

        def stage_conv(b):
            """GEMM chunks with interleaved eviction (GPSIMD) and collapse
            (DVE), then the output DMAs."""
            db = dts.pop(b)
            ab = aggs.pop(b)
            d3s = [
                dt[:, :].rearrange("p (j h t) -> p j h t", j=NJ, h=PH) for dt in db
            ]
            for cb in range(2):
                yt = py.tile([128, 2 * OF], bf16, tag="y")
                y4 = yt[:, :].rearrange("p (q h t) -> p q h t", q=2, h=H)
                for ch in range(NCH):
                    r0 = ch * CHR
                    ps = pps.tile([128, NJ * 512], f32, tag="convps")
                    for j in range(NJ):
                        out3 = ps[:, j * 512 : j * 512 + NC_].rearrange(
                            "p (h t) -> p h t", h=CHR
                        )
                        i = 0
                        for dr in range(NDR):
                            for c in range(2):
                                base = (j * NDR + dr) * CO + cb * 128
                                nc.tensor.matmul(
                                    out3[:, :, :],
                                    lhsT=ab[c][:, base : base + 128],
                                    rhs=d3s[c][:, j, r0 + dr : r0 + dr + CHR, :],
                                    start=(i == 0),
                                    stop=(i == 2 * NDR - 1),
                                )
                                i += 1
                    mt = pm.tile([128, NJ * NC_], bf16, tag="m")
                    nc.gpsimd.tensor_copy(
                        mt[:, :].rearrange("p (j n) -> p j n", j=NJ),
                        ps[:, :].rearrange("p (j n) -> p j n", j=NJ)[:, :, 0:NC_],
                    )
                    # collapse this chunk on DVE
                    m3 = mt[:, :].rearrange("p (j h t) -> p j h t", j=NJ, h=CHR)
                    s = psd.tile([128, NC_], bf16, tag="s")
                    d = psd.tile([128, NC_], bf16, tag="d")
                    s3 = s[:, :].rearrange("p (h t) -> p h t", h=CHR)
                    d3 = d[:, :].rearrange("p (h t) -> p h t", h=CHR)
                    nc.vector.tensor_tensor(s3, m3[:, 1], m3[:, 2], ALU.add)
                    nc.vector.tensor_tensor(d3, m3[:, 1], m3[:, 2], ALU.subtract)
                    nc.vector.tensor_tensor(
                        y4[:, 0, r0 : r0 + CHR, :], m3[:, 0], s3, ALU.add
                    )
                    nc.vector.tensor_tensor(
                        y4[:, 1, r0 : r0 + CHR, :], d3, m3[:, 3], ALU.subtract
                    )
                for par in range(2):
                    nc.sync.dma_start(
                        out=o_p[b, par, cb * 128 : (cb + 1) * 128, :],
                        in_=yt[:, :].rearrange("p (q f) -> p q f", q=2)[:, par, :],
                    )

        # ---- software pipeline ----
        # Preamble: x(0) first on the DMA queue, then the per-k weight
        # loads; D(0) heads the DVE queue; attention and mixing for sample
        # 0 follow as their inputs land.
        stage_load(0)
        load_weights()
        stage_pool(0)
        stage_d(0)
        stage_att_reduce(0)
        stage_mix(0)
        stage_load(1)
        stage_pool(1)
        stage_att_reduce(1)
        for b in range(BPC):
            # Window b queues: DVE: D(b+1), mix(b+1) TS+folds, collapse(b);
            # ACT: products(b+1), pooled(b+2), sigmoid(b+2) (emitted after
            # conv so GPSIMD's PAR(b+2) queues behind the evictions(b));
            # GPSIMD: evictions(b), then PAR(b+2).
            if b + 1 < BPC:
                stage_d(b + 1)
                stage_mix(b + 1)
            if b + 2 < BPC:
                stage_load(b + 2)
                stage_pool(b + 2)
            stage_conv(b)
            if b + 2 < BPC:
                stage_att_reduce(b + 2)

    nc.compile()
    return nc


def _get_nc():
    if "nc" not in _cache:
        _cache["nc"] = _build_nc()
    return _cache["nc"]


def _make_in_maps(x, att_w, weight):
    from ml_dtypes import bfloat16

    x = np.asarray(x, dtype=np.float32)
    att_w = np.asarray(att_w, dtype=np.float32)
    weight = np.asarray(weight, dtype=np.float32)
    # pad to (58, 58), split w-parity, cast bf16: (B, CI, 2, 58, 29)
    xp = np.pad(x, ((0, 0), (0, 0), (1, 1), (1, 1)))
    xh = np.empty((B_TOTAL, CI, 2, PH, TWP), dtype=bfloat16)
    xh[:, :, 0] = xp[:, :, :, 0::2]
    xh[:, :, 1] = xp[:, :, :, 1::2]
    xh = xh.reshape(B_TOTAL, CI, XF)
    # wino expert banks: (K, Cout, Cin, kh, kw) -> (Cin, K, j, kh, Cout)
    G = np.array(
        [[1, 0, 0], [0.5, 0.5, 0.5], [0.5, -0.5, 0.5], [0, 0, 1]], np.float32
    )
    wj = np.einsum("jd,koihd->ikjho", G, weight)
    wj = np.ascontiguousarray(wj).astype(bfloat16).reshape(CI, K, WF)
    awt = np.ascontiguousarray(att_w.T)  # (CI, K) f32
    return [
        {
            "x": np.ascontiguousarray(xh[i * BPC : (i + 1) * BPC]),
            "w": wj,
            "aw": awt,
        }
        for i in range(N_CORES)
    ]


def _run(x, att_w, weight, trace=False, **spmd_kwargs):
    from concourse.bass_utils import run_bass_kernel_spmd

    nc = _get_nc()
    in_maps = _make_in_maps(x, att_w, weight)
    res = run_bass_kernel_spmd(
        nc, in_maps, list(range(N_CORES)), trace=trace, **spmd_kwargs
    )
    o = np.concatenate([r["out"] for r in res.results], axis=0)
    # (B, 2, CO, H*TW) bf16 -> interleave parities, upcast
    o = o.reshape(B_TOTAL, 2, CO, H, TW).astype(np.float32)
    out = np.empty((B_TOTAL, CO, H, W), dtype=np.float32)
    out[:, :, :, 0::2] = o[:, 0]
    out[:, :, :, 1::2] = o[:, 1]
    return out, res


def kernel(x, att_w, weight):
    out, _ = _run(x, att_w, weight)
    return out


# revision 25
# speedup vs baseline: 1.3901x; 1.0845x over previous
"""MoE-routed dynamic conv kernel for Trainium2 (8 NeuronCores, SPMD).

Problem: per-sample attention (global avg pool -> 1x1 conv -> sigmoid) mixes
K=4 expert 3x3 conv kernels; each sample is convolved with its own mixed
kernel.  x: (32, 256, 56, 56), att_w: (4, 256), weight: (4, 256, 256, 3, 3).

Strategy: data parallel over batch (4 samples per core, weights replicated),
with the conv computed as 1-D Winograd F(2,3) along W (direct along H), all
tensors bf16 on the wire and in SBUF (fp32 PSUM accumulation):
  - x is zero-padded to (58, 58) on the host, cast to bf16 and split into
    even/odd column-parity planes so every device op streams step-1.
  - the K expert banks are pre-transformed on the host along kw with
    G = [[1,0,0],[.5,.5,.5],[.5,-.5,.5],[0,0,1]] -> layout (Cin, K, j, kh, Co).
  - attention (2 samples ahead): pooled sums via ACT accumulator, logits via
    a GPSIMD partition all-reduce of att_w * pooled, sigmoid on ACT.  No
    PSUM used, so the conv owns all 8 banks.
  - expert mixing (1 sample ahead): the 4 products att_k * w_k run on ACT
    as activation-copies with per-partition scale; DVE folds them with an
    in-place add chain (bf16 2x mode).
  - input transform (1 sample ahead): D_j = B^T-combo of the parity planes,
    4 DVE tensor_tensor ops per ci-block (bf16 2x).
  - GEMM: per (co-block, 14-row chunk): 4 j-planes x (3 kh x 2 ci-blocks)
    bf16 matmuls accumulate N=392 columns into bank-aligned PSUM planes
    (two 4-bank chunk tiles ping-pong).
  - eviction: one GPSIMD copy per chunk moves the 4 planes to SBUF as bf16;
    DVE collapses them (Ye = M0+M1+M2, Yo = M1-M2-M3) into parity output
    planes which DMA out as bf16; the host interleaves parities and
    upcasts to f32.

Engine-queue emission order is chosen so every engine's in-order queue sees
work in the order it becomes ready (PAR ahead of evictions on GPSIMD, mix
products ahead of nothing on ACT, D/mix ahead of collapse on DVE), keeping
the PE fed back-to-back.
"""

import sys

if "/opt/trn_rl_repo" not in sys.path:
    sys.path.insert(0, "/opt/trn_rl_repo")

import numpy as np

B_TOTAL = 32
N_CORES = 8
BPC = B_TOTAL // N_CORES  # 4
CI = 256
CO = 256
K = 4
H = W = 56
PH = 58                   # padded rows
TWP = 29                  # parity-plane cols (58/2)
TW = 28                   # output tiles per row (W/2)
NJ = 4                    # wino points
NDR = 3                   # kh taps
XF = 2 * PH * TWP         # 3364 x elems per channel (par, h, twp)
WF = NJ * NDR * CO        # 3072 wino weight elems per (k, ci): (j, dr, co)
DF = NJ * PH * TW         # 6496 D elems per channel (j, h, tw)
CHR = 14                  # oh rows per PSUM chunk
NCH = H // CHR            # 4 chunks
NC_ = CHR * TW            # 392 cols per chunk (<= 512 psum bank)
OF = H * TW               # 1568 out elems per parity per co-block

_cache = {}


def _build_nc():
    from contextlib import ExitStack

    import concourse.bacc as bacc
    import concourse.bass_isa as bass_isa
    import concourse.mybir as mybir
    import concourse.tile as tile

    f32 = mybir.dt.float32
    bf16 = mybir.dt.bfloat16
    AF = mybir.ActivationFunctionType
    ALU = mybir.AluOpType

    nc = bacc.Bacc("TRN2", target_bir_lowering=False, debug=False)
    x_p = nc.declare_dram_parameter("x", [BPC, CI, XF], bf16, isOutput=False)
    w_p = nc.declare_dram_parameter("w", [CI, K, WF], bf16, isOutput=False)
    aw_p = nc.declare_dram_parameter("aw", [CI, K], f32, isOutput=False)
    o_p = nc.declare_dram_parameter("out", [BPC, 2, CO, OF], bf16, isOutput=True)

    with ExitStack() as ctx:
        tc = ctx.enter_context(tile.TileContext(nc))
        pw = ctx.enter_context(tc.tile_pool(name="wpool", bufs=1))
        px = ctx.enter_context(tc.tile_pool(name="xpool", bufs=2))
        pd = ctx.enter_context(tc.tile_pool(name="dpool", bufs=2))
        pagg = ctx.enter_context(tc.tile_pool(name="aggpool", bufs=2))
        pu = ctx.enter_context(tc.tile_pool(name="mixu", bufs=3))
        put1 = ctx.enter_context(tc.tile_pool(name="mixt1", bufs=2))
        pm = ctx.enter_context(tc.tile_pool(name="mpool", bufs=3))
        psd = ctx.enter_context(tc.tile_pool(name="sdpool", bufs=3))
        py = ctx.enter_context(tc.tile_pool(name="ypool", bufs=1))
        psml = ctx.enter_context(tc.tile_pool(name="small", bufs=3))
        pdump = ctx.enter_context(tc.tile_pool(name="dump", bufs=1))
        pps = ctx.enter_context(tc.tile_pool(name="cpsum", bufs=2, space="PSUM"))

        # Resident replicated weights, loaded per (ci-block, expert) so the
        # first mixing products can start before the whole bank lands.
        # DMA-queue order interleaves with the x(0) load (emitted by the
        # pipeline below before load_weights is called).
        aw_sb = [
            pw.tile([128, K], f32, tag=f"aw{c}", name=f"aw{c}") for c in range(2)
        ]
        w_sb = [
            pw.tile([128, K * WF], bf16, tag=f"w{c}", name=f"wt{c}")
            for c in range(2)
        ]

        def load_weights():
            for c in range(2):
                nc.sync.dma_start(
                    out=aw_sb[c][:, :], in_=aw_p[c * 128 : (c + 1) * 128, :]
                )
            for k in range(K):
                for c in range(2):
                    nc.sync.dma_start(
                        out=w_sb[c][:, k * WF : (k + 1) * WF],
                        in_=w_p[c * 128 : (c + 1) * 128, k, :],
                    )

        xts = {}
        atts = {}
        aggs = {}
        dts = {}
        pools = {}

        def stage_load(b):
            xb = []
            for c in range(2):
                xt = px.tile([128, XF], bf16, tag=f"x{c}")
                nc.sync.dma_start(
                    out=xt[:, :], in_=x_p[b, c * 128 : (c + 1) * 128, :]
                )
                xb.append(xt)
            xts[b] = xb

        def stage_pool(b):
            """Pooled channel sums on ACT (accumulator), emitted right after
            the x DMA so it runs as the data lands."""
            xb = xts[b]
            pl = []
            for c in range(2):
                pc = psml.tile([128, 1], f32, tag=f"pooled{c}")
                dump = pdump.tile([128, XF], bf16, tag="pooldump")
                nc.scalar.activation(
                    dump[:, :], xb[c][:, :], AF.Copy, accum_out=pc[:, :]
                )
                pl.append(pc)
            pools[b] = pl

        def stage_att_reduce(b):
            """pooled -> logits via GPSIMD partition all-reduce -> sigmoid."""
            pooled = pools.pop(b)
            tka = psml.tile([128, K], f32, tag="tka")
            tkb = psml.tile([128, K], f32, tag="tkb")
            nc.vector.tensor_scalar_mul(tka[:, :], aw_sb[0][:, :], pooled[0][:, :])
            nc.vector.tensor_scalar_mul(tkb[:, :], aw_sb[1][:, :], pooled[1][:, :])
            nc.vector.tensor_tensor(tka[:, :], tka[:, :], tkb[:, :], ALU.add)
            logit = psml.tile([128, K], f32, tag="logit")
            nc.gpsimd.partition_all_reduce(
                logit[:, :], tka[:, :], 128, bass_isa.ReduceOp.add
            )
            att = psml.tile([128, K], f32, tag="att")
            nc.scalar.activation(
                att[:, :], logit[:, :], AF.Sigmoid, scale=1.0 / (H * W)
            )
            atts[b] = att

        def stage_d(b):
            """Input transform: D_j from parity planes, 4 TT per ci-block."""
            xb = xts.pop(b)
            db = []
            for c in range(2):
                x3 = xb[c][:, :].rearrange("p (q h t) -> p q h t", q=2, h=PH)
                xe0 = x3[:, 0, :, 0:TW]
                xe1 = x3[:, 0, :, 1 : TW + 1]
                xo0 = x3[:, 1, :, 0:TW]
                xo1 = x3[:, 1, :, 1 : TW + 1]
                dt = pd.tile([128, DF], bf16, tag=f"d{c}")
                d3 = dt[:, :].rearrange("p (j h t) -> p j h t", j=NJ, h=PH)
                nc.vector.tensor_tensor(d3[:, 0], xe0, xe1, ALU.subtract)
                nc.vector.tensor_tensor(d3[:, 1], xo0, xe1, ALU.add)
                nc.vector.tensor_tensor(d3[:, 2], xe1, xo0, ALU.subtract)
                nc.vector.tensor_tensor(d3[:, 3], xo0, xo1, ALU.subtract)
                db.append(dt)
            dts[b] = db

        def stage_mix(b, head=False):
            """agg_c = sum_k att_k * w_k.

            Steady state (head=False): ci-block 0's four products on ACT;
            ci-block 1 takes k0/k1 as DVE tensor_scalar (4x mode), k2/k3 on
            ACT.  DVE folds with in-place adds (bf16 2x) in readiness order.

            head=True (sample 0): the per-k weight DMAs gate everything, so
            only c0/k0..k2 go to ACT; the DMA-critical tail (k3c0 and all of
            c1) runs as DVE tensor_scalar, which is cheap enough to chase
            each arriving weight slice.
            """
            att = atts.pop(b)
            TT = nc.vector.tensor_tensor
            TS = nc.vector.tensor_scalar_mul

            def wslice(c, k):
                return w_sb[c][:, k * WF : (k + 1) * WF]

            def act_prod(dst, c, k):
                nc.scalar.activation(
                    dst[:, :], wslice(c, k), AF.Copy, scale=att[:, k : k + 1]
                )

            ag0 = pagg.tile([128, WF], bf16, tag="agg0")
            ag1 = pagg.tile([128, WF], bf16, tag="agg1")
            if head:
                u1c0 = pu.tile([128, WF], bf16, tag="u")
                u2c0 = pu.tile([128, WF], bf16, tag="u")
                act_prod(ag0, 0, 0)
                act_prod(u1c0, 0, 1)
                act_prod(u2c0, 0, 2)
                ta = put1.tile([128, WF], bf16, tag="t1")
                tb = put1.tile([128, WF], bf16, tag="t1")
                tc_ = put1.tile([128, WF], bf16, tag="t1")
                td = put1.tile([128, WF], bf16, tag="t1")
                TS(ag1[:, :], wslice(1, 0), att[:, 0:1])
                TS(ta[:, :], wslice(1, 1), att[:, 1:2])
                TT(ag1[:, :], ag1[:, :], ta[:, :], ALU.add)
                TT(ag0[:, :], ag0[:, :], u1c0[:, :], ALU.add)
                TS(tb[:, :], wslice(1, 2), att[:, 2:3])
                TT(ag1[:, :], ag1[:, :], tb[:, :], ALU.add)
                TT(ag0[:, :], ag0[:, :], u2c0[:, :], ALU.add)
                TS(tc_[:, :], wslice(0, 3), att[:, 3:4])
                TT(ag0[:, :], ag0[:, :], tc_[:, :], ALU.add)
                TS(td[:, :], wslice(1, 3), att[:, 3:4])
                TT(ag1[:, :], ag1[:, :], td[:, :], ALU.add)
            else:
                u1c0 = pu.tile([128, WF], bf16, tag="u")
                u2c0 = pu.tile([128, WF], bf16, tag="u")
                u2c1 = pu.tile([128, WF], bf16, tag="u")
                u3c0 = pu.tile([128, WF], bf16, tag="u")
                u3c1 = pu.tile([128, WF], bf16, tag="u")
                act_prod(ag0, 0, 0)
                act_prod(u1c0, 0, 1)
                act_prod(u2c0, 0, 2)
                act_prod(u2c1, 1, 2)
                act_prod(u3c0, 0, 3)
                act_prod(u3c1, 1, 3)
                t1 = put1.tile([128, WF], bf16, tag="t1")
                TS(ag1[:, :], wslice(1, 0), att[:, 0:1])
                TS(t1[:, :], wslice(1, 1), att[:, 1:2])
                TT(ag1[:, :], ag1[:, :], t1[:, :], ALU.add)
                TT(ag0[:, :], ag0[:, :], u1c0[:, :], ALU.add)
                TT(ag0[:, :], ag0[:, :], u2c0[:, :], ALU.add)
                TT(ag1[:, :], ag1[:, :], u2c1[:, :], ALU.add)
                TT(ag0[:, :], ag0[:, :], u3c0[:, :], ALU.add)
                TT(ag1[:, :], ag1[:, :], u3c1[:, :], ALU.add)
            aggs[b] = [ag0, ag1]

        def stage_conv(b):
            """GEMM chunks with interleaved eviction (GPSIMD) and collapse
            (DVE), then the output DMAs."""
            db = dts.pop(b)
            ab = aggs.pop(b)
            d3s = [
                dt[:, :].rearrange("p (j h t) -> p j h t", j=NJ, h=PH) for dt in db
            ]
            for cb in range(2):
                yt = py.tile([128, 2 * OF], bf16, tag="y")
                y4 = yt[:, :].rearrange("p (q h t) -> p q h t", q=2, h=H)
                for ch in range(NCH):
                    r0 = ch * CHR
                    ps = pps.tile([128, NJ * 512], f32, tag="convps")
                    for j in range(NJ):
                        out3 = ps[:, j * 512 : j * 512 + NC_].rearrange(
                            "p (h t) -> p h t", h=CHR
                        )
                        i = 0
                        for dr in range(NDR):
                            for c in range(2):
                                base = (j * NDR + dr) * CO + cb * 128
                                nc.tensor.matmul(
                                    out3[:, :, :],
                                    lhsT=ab[c][:, base : base + 128],
                                    rhs=d3s[c][:, j, r0 + dr : r0 + dr + CHR, :],
                                    start=(i == 0),
                                    stop=(i == 2 * NDR - 1),
                                )
                                i += 1
                    mt = pm.tile([128, NJ * NC_], bf16, tag="m")
                    nc.gpsimd.tensor_copy(
                        mt[:, :].rearrange("p (j n) -> p j n", j=NJ),
                        ps[:, :].rearrange("p (j n) -> p j n", j=NJ)[:, :, 0:NC_],
                    )
                    # collapse this chunk on DVE
                    m3 = mt[:, :].rearrange("p (j h t) -> p j h t", j=NJ, h=CHR)
                    s = psd.tile([128, NC_], bf16, tag="s")
                    d = psd.tile([128, NC_], bf16, tag="d")
                    s3 = s[:, :].rearrange("p (h t) -> p h t", h=CHR)
                    d3 = d[:, :].rearrange("p (h t) -> p h t", h=CHR)
                    nc.vector.tensor_tensor(s3, m3[:, 1], m3[:, 2], ALU.add)
                    nc.vector.tensor_tensor(d3, m3[:, 1], m3[:, 2], ALU.subtract)
                    nc.vector.tensor_tensor(
                        y4[:, 0, r0 : r0 + CHR, :], m3[:, 0], s3, ALU.add
                    )
                    nc.vector.tensor_tensor(
                        y4[:, 1, r0 : r0 + CHR, :], d3, m3[:, 3], ALU.subtract
                    )
                    if ch % 2 == 1:
                        # flush the finished half so the tail DMA is short
                        hf = ch // 2
                        half = OF // 2
                        for par in range(2):
                            nc.sync.dma_start(
                                out=o_p[
                                    b,
                                    par,
                                    cb * 128 : (cb + 1) * 128,
                                    hf * half : (hf + 1) * half,
                                ],
                                in_=yt[:, :].rearrange("p (q f) -> p q f", q=2)[
                                    :, par, hf * half : (hf + 1) * half
                                ],
                            )

        # ---- software pipeline ----
        # Preamble: x(0) first on the DMA queue, then the per-k weight
        # loads; D(0) heads the DVE queue; attention and mixing for sample
        # 0 follow as their inputs land.
        stage_load(0)
        load_weights()
        stage_pool(0)
        stage_d(0)
        stage_att_reduce(0)
        stage_mix(0, head=True)
        stage_load(1)
        stage_pool(1)
        stage_att_reduce(1)
        for b in range(BPC):
            # Window b queues: DVE: D(b+1), mix(b+1) TS+folds, collapse(b);
            # ACT: products(b+1), pooled(b+2), sigmoid(b+2) (emitted after
            # conv so GPSIMD's PAR(b+2) queues behind the evictions(b));
            # GPSIMD: evictions(b), then PAR(b+2).
            if b + 1 < BPC:
                stage_d(b + 1)
                stage_mix(b + 1)
            if b + 2 < BPC:
                stage_load(b + 2)
                stage_pool(b + 2)
            stage_conv(b)
            if b + 2 < BPC:
                stage_att_reduce(b + 2)

    nc.compile()
    return nc


def _get_nc():
    if "nc" not in _cache:
        _cache["nc"] = _build_nc()
    return _cache["nc"]


def _make_in_maps(x, att_w, weight):
    from ml_dtypes import bfloat16

    x = np.asarray(x, dtype=np.float32)
    att_w = np.asarray(att_w, dtype=np.float32)
    weight = np.asarray(weight, dtype=np.float32)
    # pad to (58, 58), split w-parity, cast bf16: (B, CI, 2, 58, 29)
    xp = np.pad(x, ((0, 0), (0, 0), (1, 1), (1, 1)))
    xh = np.empty((B_TOTAL, CI, 2, PH, TWP), dtype=bfloat16)
    xh[:, :, 0] = xp[:, :, :, 0::2]
    xh[:, :, 1] = xp[:, :, :, 1::2]
    xh = xh.reshape(B_TOTAL, CI, XF)
    # wino expert banks: (K, Cout, Cin, kh, kw) -> (Cin, K, j, kh, Cout)
    G = np.array(
        [[1, 0, 0], [0.5, 0.5, 0.5], [0.5, -0.5, 0.5], [0, 0, 1]], np.float32
    )
    wj = np.einsum("jd,koihd->ikjho", G, weight)
    wj = np.ascontiguousarray(wj).astype(bfloat16).reshape(CI, K, WF)
    awt = np.ascontiguousarray(att_w.T)  # (CI, K) f32
    return [
        {
            "x": np.ascontiguousarray(xh[i * BPC : (i + 1) * BPC]),
            "w": wj,
            "aw": awt,
        }
        for i in range(N_CORES)
    ]


def _run(x, att_w, weight, trace=False, **spmd_kwargs):
    from concourse.bass_utils import run_bass_kernel_spmd

    nc = _get_nc()
    in_maps = _make_in_maps(x, att_w, weight)
    res = run_bass_kernel_spmd(
        nc, in_maps, list(range(N_CORES)), trace=trace, **spmd_kwargs
    )
    o = np.concatenate([r["out"] for r in res.results], axis=0)
    # (B, 2, CO, H*TW) bf16 -> interleave parities, upcast
    o = o.reshape(B_TOTAL, 2, CO, H, TW).astype(np.float32)
    out = np.empty((B_TOTAL, CO, H, W), dtype=np.float32)
    out[:, :, :, 0::2] = o[:, 0]
    out[:, :, :, 1::2] = o[:, 1]
    return out, res


def kernel(x, att_w, weight):
    out, _ = _run(x, att_w, weight)
    return out


# revision 31
# speedup vs baseline: 1.4106x; 1.0147x over previous
"""MoE-routed dynamic conv kernel for Trainium2 (8 NeuronCores, SPMD).

Problem: per-sample attention (global avg pool -> 1x1 conv -> sigmoid) mixes
K=4 expert 3x3 conv kernels; each sample is convolved with its own mixed
kernel.  x: (32, 256, 56, 56), att_w: (4, 256), weight: (4, 256, 256, 3, 3).

Strategy: data parallel over batch (4 samples per core, weights replicated),
with the conv computed as 1-D Winograd F(2,3) along W (direct along H), all
tensors bf16 on the wire and in SBUF (fp32 PSUM accumulation):
  - x is zero-padded to (58, 58) on the host, cast to bf16 and split into
    even/odd column-parity planes so every device op streams step-1.
  - the K expert banks are pre-transformed on the host along kw with
    G = [[1,0,0],[.5,.5,.5],[.5,-.5,.5],[0,0,1]] -> layout (Cin, K, j, kh, Co).
  - attention (2 samples ahead): pooled sums via ACT accumulator, logits via
    a GPSIMD partition all-reduce of att_w * pooled, sigmoid on ACT.  No
    PSUM used, so the conv owns all 8 banks.
  - expert mixing (1 sample ahead): the 4 products att_k * w_k run on ACT
    as activation-copies with per-partition scale; DVE folds them with an
    in-place add chain (bf16 2x mode).
  - input transform (1 sample ahead): D_j = B^T-combo of the parity planes,
    4 DVE tensor_tensor ops per ci-block (bf16 2x).
  - GEMM: per (co-block, 14-row chunk): 4 j-planes x (3 kh x 2 ci-blocks)
    bf16 matmuls accumulate N=392 columns into bank-aligned PSUM planes
    (two 4-bank chunk tiles ping-pong).
  - eviction: one GPSIMD copy per chunk moves the 4 planes to SBUF as bf16;
    DVE collapses them (Ye = M0+M1+M2, Yo = M1-M2-M3) into parity output
    planes which DMA out as bf16; the host interleaves parities and
    upcasts to f32.

Engine-queue emission order is chosen so every engine's in-order queue sees
work in the order it becomes ready (PAR ahead of evictions on GPSIMD, mix
products ahead of nothing on ACT, D/mix ahead of collapse on DVE), keeping
the PE fed back-to-back.
"""

import sys

if "/opt/trn_rl_repo" not in sys.path:
    sys.path.insert(0, "/opt/trn_rl_repo")

import numpy as np

B_TOTAL = 32
N_CORES = 8
BPC = B_TOTAL // N_CORES  # 4
CI = 256
CO = 256
K = 4
H = W = 56
PH = 58                   # padded rows
TWP = 29                  # parity-plane cols (58/2)
TW = 28                   # output tiles per row (W/2)
NJ = 4                    # wino points
NDR = 3                   # kh taps
XF = 2 * PH * TWP         # 3364 x elems per channel (par, h, twp)
WF = NJ * NDR * CO        # 3072 wino weight elems per (k, ci): (j, dr, co)
DF = NJ * PH * TW         # 6496 D elems per channel (j, h, tw)
CHR = 14                  # oh rows per PSUM chunk
NCH = H // CHR            # 4 chunks
NC_ = CHR * TW            # 392 cols per chunk (<= 512 psum bank)
OF = H * TW               # 1568 out elems per parity per co-block

_cache = {}


def _build_nc():
    from contextlib import ExitStack

    import concourse.bacc as bacc
    import concourse.bass_isa as bass_isa
    import concourse.mybir as mybir
    import concourse.tile as tile

    f32 = mybir.dt.float32
    bf16 = mybir.dt.bfloat16
    AF = mybir.ActivationFunctionType
    ALU = mybir.AluOpType

    nc = bacc.Bacc("TRN2", target_bir_lowering=False, debug=False)
    x_p = nc.declare_dram_parameter("x", [BPC, CI, XF], bf16, isOutput=False)
    w_p = nc.declare_dram_parameter("w", [CI, K, WF], bf16, isOutput=False)
    aw_p = nc.declare_dram_parameter("aw", [CI, K], f32, isOutput=False)
    o_p = nc.declare_dram_parameter("out", [BPC, 2, CO, OF], bf16, isOutput=True)

    with ExitStack() as ctx:
        tc = ctx.enter_context(tile.TileContext(nc))
        pw = ctx.enter_context(tc.tile_pool(name="wpool", bufs=1))
        px = ctx.enter_context(tc.tile_pool(name="xpool", bufs=2))
        pd = ctx.enter_context(tc.tile_pool(name="dpool", bufs=2))
        pagg = ctx.enter_context(tc.tile_pool(name="aggpool", bufs=2))
        pu = ctx.enter_context(tc.tile_pool(name="mixu", bufs=3))
        put1 = ctx.enter_context(tc.tile_pool(name="mixt1", bufs=2))
        pm = ctx.enter_context(tc.tile_pool(name="mpool", bufs=3))
        psd = ctx.enter_context(tc.tile_pool(name="sdpool", bufs=3))
        py = ctx.enter_context(tc.tile_pool(name="ypool", bufs=1))
        psml = ctx.enter_context(tc.tile_pool(name="small", bufs=3))
        pdump = ctx.enter_context(tc.tile_pool(name="dump", bufs=1))
        pps = ctx.enter_context(tc.tile_pool(name="cpsum", bufs=2, space="PSUM"))

        # Resident replicated weights, loaded per (ci-block, expert) so the
        # first mixing products can start before the whole bank lands.
        # DMA-queue order interleaves with the x(0) load (emitted by the
        # pipeline below before load_weights is called).
        aw_sb = [
            pw.tile([128, K], f32, tag=f"aw{c}", name=f"aw{c}") for c in range(2)
        ]
        w_sb = [
            pw.tile([128, K * WF], bf16, tag=f"w{c}", name=f"wt{c}")
            for c in range(2)
        ]

        def load_weights():
            for c in range(2):
                nc.sync.dma_start(
                    out=aw_sb[c][:, :], in_=aw_p[c * 128 : (c + 1) * 128, :]
                )
            # k-major so sample 0's mixing chases the arrivals; k3 lands
            # c1-first because c1's products are the cheap DVE tensor_scalar
            # path and gate the (c1-first) first conv chunk.
            for k, c in [(0, 0), (0, 1), (1, 0), (1, 1), (2, 0), (2, 1), (3, 1), (3, 0)]:
                nc.sync.dma_start(
                    out=w_sb[c][:, k * WF : (k + 1) * WF],
                    in_=w_p[c * 128 : (c + 1) * 128, k, :],
                )

        xts = {}
        atts = {}
        aggs = {}
        dts = {}
        pools = {}

        def stage_load(b):
            xb = []
            for c in range(2):
                xt = px.tile([128, XF], bf16, tag=f"x{c}")
                nc.sync.dma_start(
                    out=xt[:, :], in_=x_p[b, c * 128 : (c + 1) * 128, :]
                )
                xb.append(xt)
            xts[b] = xb

        def stage_pool(b):
            """Pooled channel sums on ACT (accumulator), emitted right after
            the x DMA so it runs as the data lands."""
            xb = xts[b]
            pl = []
            for c in range(2):
                pc = psml.tile([128, 1], f32, tag=f"pooled{c}")
                dump = pdump.tile([128, XF], bf16, tag="pooldump")
                nc.scalar.activation(
                    dump[:, :], xb[c][:, :], AF.Copy, accum_out=pc[:, :]
                )
                pl.append(pc)
            pools[b] = pl

        def stage_att_reduce(b):
            """pooled -> logits via GPSIMD partition all-reduce -> sigmoid."""
            pooled = pools.pop(b)
            tka = psml.tile([128, K], f32, tag="tka")
            tkb = psml.tile([128, K], f32, tag="tkb")
            nc.vector.tensor_scalar_mul(tka[:, :], aw_sb[0][:, :], pooled[0][:, :])
            nc.vector.tensor_scalar_mul(tkb[:, :], aw_sb[1][:, :], pooled[1][:, :])
            nc.vector.tensor_tensor(tka[:, :], tka[:, :], tkb[:, :], ALU.add)
            logit = psml.tile([128, K], f32, tag="logit")
            nc.gpsimd.partition_all_reduce(
                logit[:, :], tka[:, :], 128, bass_isa.ReduceOp.add
            )
            att = psml.tile([128, K], f32, tag="att")
            nc.scalar.activation(
                att[:, :], logit[:, :], AF.Sigmoid, scale=1.0 / (H * W)
            )
            atts[b] = att

        def stage_d(b, mid_hook=None):
            """Input transform: D_j from parity planes, 4 TT per ci-block."""
            xb = xts.pop(b)
            db = []
            for c in range(2):
                if c == 1 and mid_hook is not None:
                    mid_hook()
                x3 = xb[c][:, :].rearrange("p (q h t) -> p q h t", q=2, h=PH)
                xe0 = x3[:, 0, :, 0:TW]
                xe1 = x3[:, 0, :, 1 : TW + 1]
                xo0 = x3[:, 1, :, 0:TW]
                xo1 = x3[:, 1, :, 1 : TW + 1]
                dt = pd.tile([128, DF], bf16, tag=f"d{c}")
                d3 = dt[:, :].rearrange("p (j h t) -> p j h t", j=NJ, h=PH)
                nc.vector.tensor_tensor(d3[:, 0], xe0, xe1, ALU.subtract)
                nc.vector.tensor_tensor(d3[:, 1], xo0, xe1, ALU.add)
                nc.vector.tensor_tensor(d3[:, 2], xe1, xo0, ALU.subtract)
                nc.vector.tensor_tensor(d3[:, 3], xo0, xo1, ALU.subtract)
                db.append(dt)
            dts[b] = db

        def stage_mix(b, head=False):
            """agg_c = sum_k att_k * w_k.

            Steady state (head=False): ci-block 0's four products on ACT;
            ci-block 1 takes k0/k1 as DVE tensor_scalar (4x mode), k2/k3 on
            ACT.  DVE folds with in-place adds (bf16 2x) in readiness order.

            head=True (sample 0): the per-k weight DMAs gate everything, so
            only c0/k0..k2 go to ACT; the DMA-critical tail (k3c0 and all of
            c1) runs as DVE tensor_scalar, which is cheap enough to chase
            each arriving weight slice.
            """
            att = atts.pop(b)
            TT = nc.vector.tensor_tensor
            TS = nc.vector.tensor_scalar_mul

            def wslice(c, k):
                return w_sb[c][:, k * WF : (k + 1) * WF]

            def act_prod(dst, c, k):
                nc.scalar.activation(
                    dst[:, :], wslice(c, k), AF.Copy, scale=att[:, k : k + 1]
                )

            ag0 = pagg.tile([128, WF], bf16, tag="agg0")
            ag1 = pagg.tile([128, WF], bf16, tag="agg1")
            if head:
                u1c0 = pu.tile([128, WF], bf16, tag="u")
                u2c0 = pu.tile([128, WF], bf16, tag="u")
                act_prod(ag0, 0, 0)
                act_prod(u1c0, 0, 1)
                act_prod(u2c0, 0, 2)
                ta = put1.tile([128, WF], bf16, tag="t1")
                tb = put1.tile([128, WF], bf16, tag="t1")
                tc_ = put1.tile([128, WF], bf16, tag="t1")
                td = put1.tile([128, WF], bf16, tag="t1")
                TS(ag1[:, :], wslice(1, 0), att[:, 0:1])
                TS(ta[:, :], wslice(1, 1), att[:, 1:2])
                TT(ag1[:, :], ag1[:, :], ta[:, :], ALU.add)
                TT(ag0[:, :], ag0[:, :], u1c0[:, :], ALU.add)
                TS(tb[:, :], wslice(1, 2), att[:, 2:3])
                TT(ag1[:, :], ag1[:, :], tb[:, :], ALU.add)
                TT(ag0[:, :], ag0[:, :], u2c0[:, :], ALU.add)
                TS(td[:, :], wslice(1, 3), att[:, 3:4])
                TT(ag1[:, :], ag1[:, :], td[:, :], ALU.add)
                TS(tc_[:, :], wslice(0, 3), att[:, 3:4])
                TT(ag0[:, :], ag0[:, :], tc_[:, :], ALU.add)
            else:
                u1c0 = pu.tile([128, WF], bf16, tag="u")
                u2c0 = pu.tile([128, WF], bf16, tag="u")
                u2c1 = pu.tile([128, WF], bf16, tag="u")
                u3c0 = pu.tile([128, WF], bf16, tag="u")
                u3c1 = pu.tile([128, WF], bf16, tag="u")
                act_prod(ag0, 0, 0)
                act_prod(u1c0, 0, 1)
                act_prod(u2c0, 0, 2)
                act_prod(u2c1, 1, 2)
                act_prod(u3c0, 0, 3)
                act_prod(u3c1, 1, 3)
                t1 = put1.tile([128, WF], bf16, tag="t1")
                TS(ag1[:, :], wslice(1, 0), att[:, 0:1])
                TS(t1[:, :], wslice(1, 1), att[:, 1:2])
                TT(ag1[:, :], ag1[:, :], t1[:, :], ALU.add)
                TT(ag0[:, :], ag0[:, :], u1c0[:, :], ALU.add)
                TT(ag0[:, :], ag0[:, :], u2c0[:, :], ALU.add)
                TT(ag1[:, :], ag1[:, :], u2c1[:, :], ALU.add)
                TT(ag0[:, :], ag0[:, :], u3c0[:, :], ALU.add)
                TT(ag1[:, :], ag1[:, :], u3c1[:, :], ALU.add)
            aggs[b] = [ag0, ag1]

        def stage_conv(b):
            """GEMM chunks with interleaved eviction (GPSIMD) and collapse
            (DVE), then the output DMAs."""
            db = dts.pop(b)
            ab = aggs.pop(b)
            d3s = [
                dt[:, :].rearrange("p (j h t) -> p j h t", j=NJ, h=PH) for dt in db
            ]
            for cb in range(2):
                yt = py.tile([128, 2 * OF], bf16, tag="y")
                y4 = yt[:, :].rearrange("p (q h t) -> p q h t", q=2, h=H)
                for ch in range(NCH):
                    r0 = ch * CHR
                    ps = pps.tile([128, NJ * 512], f32, tag="convps")
                    for j in range(NJ):
                        out3 = ps[:, j * 512 : j * 512 + NC_].rearrange(
                            "p (h t) -> p h t", h=CHR
                        )
                        i = 0
                        for c in (1, 0):  # ag1 lands first in the preamble
                            for dr in range(NDR):
                                base = (j * NDR + dr) * CO + cb * 128
                                nc.tensor.matmul(
                                    out3[:, :, :],
                                    lhsT=ab[c][:, base : base + 128],
                                    rhs=d3s[c][:, j, r0 + dr : r0 + dr + CHR, :],
                                    start=(i == 0),
                                    stop=(i == 2 * NDR - 1),
                                )
                                i += 1
                    mt = pm.tile([128, NJ * NC_], bf16, tag="m")
                    nc.gpsimd.tensor_copy(
                        mt[:, :].rearrange("p (j n) -> p j n", j=NJ),
                        ps[:, :].rearrange("p (j n) -> p j n", j=NJ)[:, :, 0:NC_],
                    )
                    # collapse this chunk on DVE
                    m3 = mt[:, :].rearrange("p (j h t) -> p j h t", j=NJ, h=CHR)
                    s = psd.tile([128, NC_], bf16, tag="s")
                    d = psd.tile([128, NC_], bf16, tag="d")
                    s3 = s[:, :].rearrange("p (h t) -> p h t", h=CHR)
                    d3 = d[:, :].rearrange("p (h t) -> p h t", h=CHR)
                    # s/d on GPSIMD (right after its eviction) frees DVE slack
                    nc.gpsimd.tensor_tensor(s3, m3[:, 1], m3[:, 2], ALU.add)
                    nc.gpsimd.tensor_tensor(d3, m3[:, 1], m3[:, 2], ALU.subtract)
                    nc.vector.tensor_tensor(
                        y4[:, 0, r0 : r0 + CHR, :], m3[:, 0], s3, ALU.add
                    )
                    nc.vector.tensor_tensor(
                        y4[:, 1, r0 : r0 + CHR, :], d3, m3[:, 3], ALU.subtract
                    )
                    if ch % 2 == 1:
                        # flush the finished half so the tail DMA is short
                        hf = ch // 2
                        half = OF // 2
                        for par in range(2):
                            nc.sync.dma_start(
                                out=o_p[
                                    b,
                                    par,
                                    cb * 128 : (cb + 1) * 128,
                                    hf * half : (hf + 1) * half,
                                ],
                                in_=yt[:, :].rearrange("p (q f) -> p q f", q=2)[
                                    :, par, hf * half : (hf + 1) * half
                                ],
                            )

        # ---- software pipeline ----
        # Preamble: x(0) first on the DMA queue, then the per-k weight
        # loads; D(0) heads the DVE queue; attention and mixing for sample
        # 0 follow as their inputs land.
        stage_load(0)
        load_weights()
        stage_pool(0)
        stage_d(0, mid_hook=lambda: stage_att_reduce(0))
        stage_mix(0, head=True)
        stage_load(1)
        stage_pool(1)
        stage_att_reduce(1)
        for b in range(BPC):
            # Window b queues: DVE: D(b+1), mix(b+1) TS+folds, collapse(b);
            # ACT: products(b+1), pooled(b+2), sigmoid(b+2) (emitted after
            # conv so GPSIMD's PAR(b+2) queues behind the evictions(b));
            # GPSIMD: evictions(b), then PAR(b+2).
            if b + 1 < BPC:
                stage_d(b + 1)
                stage_mix(b + 1)
            if b + 2 < BPC:
                stage_load(b + 2)
                stage_pool(b + 2)
            stage_conv(b)
            if b + 2 < BPC:
                stage_att_reduce(b + 2)

    nc.compile()
    return nc


def _get_nc():
    if "nc" not in _cache:
        _cache["nc"] = _build_nc()
    return _cache["nc"]


def _make_in_maps(x, att_w, weight):
    from ml_dtypes import bfloat16

    x = np.asarray(x, dtype=np.float32)
    att_w = np.asarray(att_w, dtype=np.float32)
    weight = np.asarray(weight, dtype=np.float32)
    # pad to (58, 58), split w-parity, cast bf16: (B, CI, 2, 58, 29)
    xp = np.pad(x, ((0, 0), (0, 0), (1, 1), (1, 1)))
    xh = np.empty((B_TOTAL, CI, 2, PH, TWP), dtype=bfloat16)
    xh[:, :, 0] = xp[:, :, :, 0::2]
    xh[:, :, 1] = xp[:, :, :, 1::2]
    xh = xh.reshape(B_TOTAL, CI, XF)
    # wino expert banks: (K, Cout, Cin, kh, kw) -> (Cin, K, j, kh, Cout)
    G = np.array(
        [[1, 0, 0], [0.5, 0.5, 0.5], [0.5, -0.5, 0.5], [0, 0, 1]], np.float32
    )
    wj = np.einsum("jd,koihd->ikjho", G, weight)
    wj = np.ascontiguousarray(wj).astype(bfloat16).reshape(CI, K, WF)
    awt = np.ascontiguousarray(att_w.T)  # (CI, K) f32
    return [
        {
            "x": np.ascontiguousarray(xh[i * BPC : (i + 1) * BPC]),
            "w": wj,
            "aw": awt,
        }
        for i in range(N_CORES)
    ]


def _run(x, att_w, weight, trace=False, **spmd_kwargs):
    from concourse.bass_utils import run_bass_kernel_spmd

    nc = _get_nc()
    in_maps = _make_in_maps(x, att_w, weight)
    res = run_bass_kernel_spmd(
        nc, in_maps, list(range(N_CORES)), trace=trace, **spmd_kwargs
    )
    o = np.concatenate([r["out"] for r in res.results], axis=0)
    # (B, 2, CO, H*TW) bf16 -> interleave parities, upcast
    o = o.reshape(B_TOTAL, 2, CO, H, TW).astype(np.float32)
    out = np.empty((B_TOTAL, CO, H, W), dtype=np.float32)
    out[:, :, :, 0::2] = o[:, 0]
    out[:, :, :, 1::2] = o[:, 1]
    return out, res


def kernel(x, att_w, weight):
    out, _ = _run(x, att_w, weight)
    return out
